# revision 8
# baseline (speedup 1.0000x reference)
"""BitBottleneck (ternary-quantized 3x3 conv x2 + BN + SiLU + residual) on 8 trn2 cores.

Strategy (v2, fp8 DoubleRow):
  - Data-parallel over batch: 32 images -> 4 per core, no collectives.
  - Ternary weights {-1,0,+1} are exact in fp8 e4m3; the per-tensor scale s
    folds into the BN scale. Activations are quantized to e4m3 (x on host,
    h on-chip via ScalarE's RNE cast on the SiLU write). Measured end-to-end
    rel err 1.89e-2 (budget 2e-2); the conv arithmetic itself is exact.
  - PE runs fp8 DoubleRow (2 MACs/cell/cycle): conv1 pairs the two cin-128
    blocks (contraction 256 per stream); conv2 pairs taps — 3 horizontal
    pairs (stride 1), 1 vertical pair (stride 64), 1 normal tap — 4 DR + 1
    plain matmul per output block instead of 9.
  - Weight-stationary schedule: for each weight tile, stream all 7 row-block
    PSUM banks (one image's conv output) before switching tiles, so each
    LDWEIGHTS (DoubleRow disables fast-weight-load) is amortized over 7
    matmuls and hides behind streaming.
  - x/h planes are padded to row pitch 64 so every tap is a strided view and
    DR pair strides meet the %16 LDWEIGHTS constraint where it applies.
  - Residual uses a separate fp16(x) plane (2.1e-4 error, negligible here);
    outputs are stored fp16 and upcast on host.
"""

import sys

if "/opt/trn_rl_repo" not in sys.path:
    sys.path.insert(0, "/opt/trn_rl_repo")

import numpy as np

B, C, H, W = 32, 256, 56, 56
HID = 128
NCORES = 8
IPC = B // NCORES  # images per core
PITCH = 64  # padded row pitch (58 cols used)
HP = 58  # padded rows
PLANE = HP * PITCH  # 3712 bytes/partition per cin block (fp8)
RB = 8  # output rows per row-block
NRB = H // RB  # 7
NMM = RB * W  # 448 matmul free dim
BN_EPS = 1e-5
Q_EPS = 1e-5

_CACHE = {}


def _build_nc():
    if "nc" in _CACHE:
        return _CACHE["nc"]

    import concourse.bass as bass
    import concourse.mybir as mybir
    import concourse.tile as tile
    from concourse import bacc
    from concourse.bass import AP

    f32 = mybir.dt.float32
    f16 = mybir.dt.float16
    f8 = mybir.dt.float8e4
    SILU = mybir.ActivationFunctionType.Silu
    DR = mybir.MatmulPerfMode.DoubleRow

    nc = bacc.Bacc("TRN2", target_bir_lowering=False, debug=False)

    xp8 = nc.dram_tensor("xp8", [IPC, 2, 128, PLANE], f8, kind="ExternalInput")
    xr = nc.dram_tensor("xr", [IPC, 128, 2 * H * W], f16, kind="ExternalInput")
    w1t = nc.dram_tensor("w1t", [128, 9 * 256], f8, kind="ExternalInput")
    w2t = nc.dram_tensor("w2t", [128, 2 * 1152], f8, kind="ExternalInput")
    ab1 = nc.dram_tensor("ab1", [128, 2], f32, kind="ExternalInput")
    ab2 = nc.dram_tensor("ab2", [128, 4], f32, kind="ExternalInput")
    y = nc.dram_tensor("y", [IPC, 2, 128, H * W], f16, kind="ExternalOutput")

    def pv(base, off, dims):
        """Strided (possibly overlapping) free view: dims = [(stride, n), ...]."""
        return AP(base.tensor, base.offset + off, [list(base.ap[0])] + [[s, n] for s, n in dims])

    # conv2 tap plan per cout block: (kind, ap-offset-fn, pair-stride)
    #   3 horizontal DR pairs (ky,0)+(ky,1), 1 vertical DR pair (0,2)+(1,2),
    #   1 normal tap (2,2)
    C2PLAN = [
        ("dr", lambda rb: (8 * rb + 0) * PITCH + 0, 1),
        ("dr", lambda rb: (8 * rb + 1) * PITCH + 0, 1),
        ("dr", lambda rb: (8 * rb + 2) * PITCH + 0, 1),
        ("dr", lambda rb: (8 * rb + 0) * PITCH + 2, PITCH),
        ("n", lambda rb: (8 * rb + 2) * PITCH + 2, None),
    ]

    with tile.TileContext(nc) as tc:
        with (
            tc.tile_pool(name="consts", bufs=1) as cpool,
            tc.tile_pool(name="xin", bufs=4) as xpool,
            tc.tile_pool(name="hbuf", bufs=1) as hpool,
            tc.tile_pool(name="outs", bufs=6) as opool,
            tc.tile_pool(name="ps", bufs=8, space=bass.MemorySpace.PSUM) as pspool,
        ):
            # The scalar DGE queue arms ~6us before sync/gpsimd, so it carries
            # everything the first image needs: W1, BN consts, then x8 of
            # image 0 (W2 and XR0 follow -- not needed until conv2).
            W1 = cpool.tile([128, 9 * 256], f8, tag="W1")
            nc.scalar.dma_start(W1[:], w1t[:, :])
            AB1 = cpool.tile([128, 2], f32, tag="AB1")
            nc.scalar.dma_start(AB1[:], ab1[:, :])
            AB2 = cpool.tile([128, 4], f32, tag="AB2")
            nc.scalar.dma_start(AB2[:], ab2[:, :])
            X80 = xpool.tile([128, 2 * PLANE], f8, tag="X8", name="X8_0")
            for blk in range(2):
                nc.scalar.dma_start(
                    X80[:, blk * PLANE : (blk + 1) * PLANE], xp8[0, blk, :, :]
                )
            W2 = cpool.tile([128, 2 * 1152], f8, tag="W2")
            nc.scalar.dma_start(W2[:], w2t[:, :])
            XR0 = xpool.tile([128, 2 * H * W], f16, tag="XR", name="XR_0")
            nc.sync.dma_start(XR0[:], xr[0, :, :])

            # h ping-pong tiles; zero the whole plane once (border rows/cols
            # are never rewritten; interior is). ScalarE Silu with scale=0
            # writes exact fp8 zeros.
            zrow = cpool.tile([128, PITCH * 2], f32, tag="zrow")
            nc.gpsimd.memset(zrow[:], 0.0)
            hts = []
            for j in range(2):
                ht = hpool.tile([128, PLANE], f8, tag=f"h{j}")
                hv = ht[:].rearrange("p (r c) -> p r c", r=HP, c=PITCH)
                for dst, n in (
                    (hv[:, 0, :], PITCH),
                    (hv[:, HP - 1, :], PITCH),
                    (hv[:, 1 : HP - 1, 0:1], HP - 2),
                    (hv[:, 1 : HP - 1, 57:58], HP - 2),
                ):
                    nc.scalar.activation(dst, zrow[:, :n], SILU, bias=0.0, scale=0.0)
                hts.append(ht)

            # PE warm-up: junk matmuls during the ~9us DMA queue-arming dead
            # time, so the HAM clock gate reaches K=8/8 (2.4GHz) before the
            # first real matmul instead of running the first ~3.4us at 1.2GHz.
            junk = cpool.tile([128, NMM], mybir.dt.float16, tag="junk")
            nc.gpsimd.memset(junk[:], 0.0)
            for _ in range(14):
                pw = pspool.tile([128, NMM], f32, tag="ps")
                nc.tensor.matmul(pw[:], junk[:, :128], junk[:], start=True, stop=True)

            for img in range(IPC):
                if img == 0:
                    X8, XR = X80, XR0
                else:
                    X8 = xpool.tile([128, 2 * PLANE], f8, tag="X8", name=f"X8_{img}")
                    XR = xpool.tile([128, 2 * H * W], f16, tag="XR", name=f"XR_{img}")
                    for blk in range(2):
                        eng = nc.sync if blk == 0 else nc.gpsimd
                        eng.dma_start(
                            X8[:, blk * PLANE : (blk + 1) * PLANE], xp8[img, blk, :, :]
                        )
                    (nc.gpsimd if img % 2 else nc.sync).dma_start(XR[:], xr[img, :, :])

                X8b = X8[:]
                XRv = XR[:].rearrange("p (b r c) -> p b r c", b=2, r=H, c=W)
                ht = hts[img % 2]
                hb = ht[:]
                hv = ht[:].rearrange("p (r c) -> p r c", r=HP, c=PITCH)

                # conv1: 256 -> 128. Weight-stationary: tap-outer over the 7
                # row-block PSUM banks; DR pairs the two cin blocks.
                ps1 = [pspool.tile([128, NMM], f32, tag="ps", name=f"ps1_{img}_{rb}") for rb in range(NRB)]
                for t in range(9):
                    ky, kx = t // 3, t % 3
                    wt = W1[:, t * 256 : (t + 1) * 256].rearrange("p (i m) -> p i m", i=2)
                    for rb in range(NRB):
                        rhs = pv(X8b, (8 * rb + ky) * PITCH + kx, [(PLANE, 2), (PITCH, 8), (1, 56)])
                        nc.tensor.matmul(
                            ps1[rb][:], wt, rhs, start=(t == 0), stop=(t == 8), perf_mode=DR
                        )
                # BN+SiLU into padded h interior (fp8 RNE write)
                for rb in range(NRB):
                    nc.scalar.activation(
                        hv[:, 1 + RB * rb : 1 + RB * rb + RB, 1:57],
                        ps1[rb][:],
                        SILU,
                        bias=AB1[:, 1:2],
                        scale=AB1[:, 0:1],
                    )

                # conv2: 128 -> 256 (two cout blocks), weight-stationary
                for cb in range(2):
                    ps2 = [pspool.tile([128, NMM], f32, tag="ps", name=f"ps2_{img}_{cb}_{rb}") for rb in range(NRB)]
                    for k, (kind, offn, pstride) in enumerate(C2PLAN):
                        base = cb * 1152 + k * 256
                        if kind == "dr":
                            wt = W2[:, base : base + 256].rearrange("p (i m) -> p i m", i=2)
                        else:
                            wt = W2[:, base : base + 128]
                        for rb in range(NRB):
                            if kind == "dr":
                                rhs = pv(hb, offn(rb), [(pstride, 2), (PITCH, 8), (1, 56)])
                                nc.tensor.matmul(
                                    ps2[rb][:], wt, rhs,
                                    start=(k == 0), stop=(k == 4), perf_mode=DR,
                                )
                            else:
                                rhs = pv(hb, offn(rb), [(PITCH, 8), (1, 56)])
                                nc.tensor.matmul(
                                    ps2[rb][:], wt, rhs, start=(k == 0), stop=(k == 4)
                                )
                    for rb in range(NRB):
                        st = opool.tile([128, NMM], f16, tag="st")
                        nc.scalar.activation(
                            st[:],
                            ps2[rb][:],
                            SILU,
                            bias=AB2[:, 2 * cb + 1 : 2 * cb + 2],
                            scale=AB2[:, 2 * cb : 2 * cb + 1],
                        )
                        ot = opool.tile([128, NMM], f16, tag="ot")
                        nc.vector.tensor_add(
                            ot[:], st[:], XRv[:, cb, RB * rb : RB * rb + RB, :]
                        )
                        # stores ride gpsimd; for the last image the other
                        # queues are idle, so spread stores to shorten the
                        # final flush the exit drain waits on
                        if img == IPC - 1:
                            seng = (nc.gpsimd, nc.sync, nc.scalar)[(rb * 2 + cb) % 3]
                        else:
                            seng = nc.gpsimd
                        seng.dma_start(y[img, cb, :, rb * NMM : (rb + 1) * NMM], ot[:])

    nc.compile()
    _CACHE["nc"] = nc
    return nc


def _quant_ternary(w):
    """Match jnp: s = max(median(|w|), Q_EPS); t = clip(round(w/s), -1, 1)."""
    w = np.asarray(w, np.float32)
    s = np.float32(np.median(np.abs(w)))
    s = np.maximum(s, np.float32(Q_EPS))
    t = np.clip(np.round(w / s), np.float32(-1.0), np.float32(1.0)).astype(np.float32)
    return s, t


def prepare_inputs(x, w1, g1, b1, m1, v1, w2, g2, b2, m2, v2):
    """Host-side prep: quantize+fold weights, pad/cast x, build per-core in_maps."""
    import ml_dtypes

    E4M3 = np.dtype(ml_dtypes.float8_e4m3fn)
    F16 = np.dtype(np.float16)

    x = np.asarray(x, np.float32)

    s1, t1 = _quant_ternary(w1)
    s2, t2 = _quant_ternary(w2)

    inv1 = np.asarray(g1, np.float32) / np.sqrt(np.asarray(v1, np.float32) + np.float32(BN_EPS))
    a1 = (s1 * inv1).astype(np.float32)
    c1 = (np.asarray(b1, np.float32) - np.asarray(m1, np.float32) * inv1).astype(np.float32)
    inv2 = np.asarray(g2, np.float32) / np.sqrt(np.asarray(v2, np.float32) + np.float32(BN_EPS))
    a2 = (s2 * inv2).astype(np.float32)
    c2 = (np.asarray(b2, np.float32) - np.asarray(m2, np.float32) * inv2).astype(np.float32)

    ab1 = np.stack([a1, c1], axis=1).astype(np.float32)  # [128, 2]
    a2b = a2.reshape(2, 128)
    c2b = c2.reshape(2, 128)
    ab2 = np.stack([a2b[0], c2b[0], a2b[1], c2b[1]], axis=1).astype(np.float32)  # [128,4]

    # conv1 DR weight tiles: [cin_p, tap, pair(blk), cout]
    # value = t1[cout, blk*128 + cin_p, ky, kx]
    w1t = (
        t1.reshape(HID, 2, 128, 3, 3)  # [m, i, p, ky, kx]
        .transpose(2, 3, 4, 1, 0)      # [p, ky, kx, i, m]
        .reshape(128, 9 * 256)
    ).astype(E4M3)

    # conv2 tiles per cout block: 3 horizontal DR pairs (ky,0)+(ky,1),
    # 1 vertical DR pair (0,2)+(1,2), 1 normal (2,2)
    t2b = t2.reshape(2, 128, 128, 3, 3)  # [cb, m, p, ky, kx]
    w2arr = np.zeros((128, 2, 1152), np.float32)  # [p, cb, cols]
    for cb in range(2):
        col = 0
        for ky in range(3):  # horizontal pairs
            for i, kx in enumerate((0, 1)):
                w2arr[:, cb, col : col + 128] = t2b[cb, :, :, ky, kx].T
                col += 128
        for i, (ky, kx) in enumerate(((0, 2), (1, 2))):  # vertical pair
            w2arr[:, cb, col : col + 128] = t2b[cb, :, :, ky, kx].T
            col += 128
        w2arr[:, cb, col : col + 128] = t2b[cb, :, :, 2, 2].T
    w2t = w2arr.reshape(128, 2 * 1152).astype(E4M3)

    # x planes: fp8 padded (pitch 64) for matmuls, fp16 unpadded for residual
    xpad = np.zeros((B, C, HP, PITCH), E4M3)
    xpad[:, :, 1 : 1 + H, 1 : 1 + W] = x.astype(E4M3)
    xp8 = xpad.reshape(NCORES, IPC, 2, 128, PLANE)
    xr = (
        x.astype(F16)
        .reshape(NCORES, IPC, 2, 128, H * W)
        .transpose(0, 1, 3, 2, 4)
        .reshape(NCORES, IPC, 128, 2 * H * W)
    )

    in_maps = []
    for c in range(NCORES):
        in_maps.append(
            {
                "xp8": np.ascontiguousarray(xp8[c]),
                "xr": np.ascontiguousarray(xr[c]),
                "w1t": w1t,
                "w2t": w2t,
                "ab1": ab1,
                "ab2": ab2,
            }
        )
    return in_maps


def assemble_output(per_core_results):
    ys = np.stack([r["y"] for r in per_core_results])  # [8, IPC, 2, 128, H*W]
    return ys.astype(np.float32).reshape(B, C, H, W)


def run_spmd(in_maps, **kwargs):
    from concourse.bass_utils import run_bass_kernel_spmd

    nc = _build_nc()
    return run_bass_kernel_spmd(nc, in_maps, core_ids=list(range(NCORES)), **kwargs)


def kernel(**inputs):
    in_maps = prepare_inputs(**inputs)
    res = run_spmd(in_maps)
    return assemble_output(res.results)


# revision 9
# speedup vs baseline: 1.2134x; 1.2134x over previous
"""BitBottleneck (ternary-quantized 3x3 conv x2 + BN + SiLU + residual) on 8 trn2 cores.

Strategy (v2, fp8 DoubleRow):
  - Data-parallel over batch: 32 images -> 4 per core, no collectives.
  - Ternary weights {-1,0,+1} are exact in fp8 e4m3; the per-tensor scale s
    folds into the BN scale. Activations are quantized to e4m3 (x on host,
    h on-chip via ScalarE's RNE cast on the SiLU write). Measured end-to-end
    rel err 1.89e-2 (budget 2e-2); the conv arithmetic itself is exact.
  - PE runs fp8 DoubleRow (2 MACs/cell/cycle): conv1 pairs the two cin-128
    blocks (contraction 256 per stream); conv2 pairs taps — 3 horizontal
    pairs (stride 1), 1 vertical pair (stride 64), 1 normal tap — 4 DR + 1
    plain matmul per output block instead of 9.
  - Weight-stationary schedule: for each weight tile, stream all 7 row-block
    PSUM banks (one image's conv output) before switching tiles, so each
    LDWEIGHTS (DoubleRow disables fast-weight-load) is amortized over 7
    matmuls and hides behind streaming.
  - x/h planes are padded to row pitch 64 so every tap is a strided view and
    DR pair strides meet the %16 LDWEIGHTS constraint where it applies.
  - Residual uses a separate fp16(x) plane (2.1e-4 error, negligible here);
    outputs are stored fp16 and upcast on host.
"""

import sys

if "/opt/trn_rl_repo" not in sys.path:
    sys.path.insert(0, "/opt/trn_rl_repo")

import numpy as np

B, C, H, W = 32, 256, 56, 56
HID = 128
NCORES = 8
IPC = B // NCORES  # images per core
PITCH = 64  # padded row pitch (58 cols used)
HP = 58  # padded rows
PLANE = HP * PITCH  # 3712 bytes/partition per cin block (fp8)
RB = 8  # output rows per row-block
NRB = H // RB  # 7
NMM = RB * W  # 448 matmul free dim
BN_EPS = 1e-5
Q_EPS = 1e-5

_CACHE = {}


def _build_nc():
    if "nc" in _CACHE:
        return _CACHE["nc"]

    import concourse.bass as bass
    import concourse.mybir as mybir
    import concourse.tile as tile
    from concourse import bacc
    from concourse.bass import AP

    f32 = mybir.dt.float32
    f16 = mybir.dt.float16
    f8 = mybir.dt.float8e4
    SILU = mybir.ActivationFunctionType.Silu
    DR = mybir.MatmulPerfMode.DoubleRow

    nc = bacc.Bacc("TRN2", target_bir_lowering=False, debug=False)

    xp8 = nc.dram_tensor("xp8", [IPC, 2, 128, PLANE], f8, kind="ExternalInput")
    xr = nc.dram_tensor("xr", [IPC, 128, 2 * H * W], f16, kind="ExternalInput")
    w1t = nc.dram_tensor("w1t", [128, 9 * 256], f8, kind="ExternalInput")
    w2t = nc.dram_tensor("w2t", [128, 2 * 1152], f8, kind="ExternalInput")
    ab1 = nc.dram_tensor("ab1", [128, 2], f32, kind="ExternalInput")
    ab2 = nc.dram_tensor("ab2", [128, 4], f32, kind="ExternalInput")
    y = nc.dram_tensor("y", [IPC, 2, 128, H * W], f16, kind="ExternalOutput")

    def pv(base, off, dims):
        """Strided (possibly overlapping) free view: dims = [(stride, n), ...]."""
        return AP(base.tensor, base.offset + off, [list(base.ap[0])] + [[s, n] for s, n in dims])

    # conv2 tap plan per cout block: (kind, ap-offset-fn, pair-stride)
    #   3 horizontal DR pairs (ky,0)+(ky,1), 1 vertical DR pair (0,2)+(1,2),
    #   1 normal tap (2,2)
    C2PLAN = [
        ("dr", lambda rb: (8 * rb + 0) * PITCH + 0, 1),
        ("dr", lambda rb: (8 * rb + 1) * PITCH + 0, 1),
        ("dr", lambda rb: (8 * rb + 2) * PITCH + 0, 1),
        ("dr", lambda rb: (8 * rb + 0) * PITCH + 2, PITCH),
        ("n", lambda rb: (8 * rb + 2) * PITCH + 2, None),
    ]

    with tile.TileContext(nc) as tc:
        with (
            tc.tile_pool(name="consts", bufs=1) as cpool,
            tc.tile_pool(name="xin", bufs=4) as xpool,
            tc.tile_pool(name="hbuf", bufs=1) as hpool,
            tc.tile_pool(name="outs", bufs=6) as opool,
            tc.tile_pool(name="ps", bufs=8, space=bass.MemorySpace.PSUM) as pspool,
        ):
            # weights/consts on the scalar DGE queue (it arms early but is
            # low-bandwidth -- bulk image data stays on sync/gpsimd).
            W1 = cpool.tile([128, 9 * 256], f8, tag="W1")
            nc.scalar.dma_start(W1[:], w1t[:, :])
            AB1 = cpool.tile([128, 2], f32, tag="AB1")
            nc.scalar.dma_start(AB1[:], ab1[:, :])
            AB2 = cpool.tile([128, 4], f32, tag="AB2")
            nc.scalar.dma_start(AB2[:], ab2[:, :])
            W2 = cpool.tile([128, 2 * 1152], f8, tag="W2")
            nc.scalar.dma_start(W2[:], w2t[:, :])
            # first image: chunk rows across both queues so conv1 can start
            # as soon as the plane lands; XR0 follows (not needed until conv2)
            X80 = xpool.tile([128, 2 * PLANE], f8, tag="X8", name="X8_0")
            XCH = [(0, 15), (15, 30), (30, 44), (44, HP)]
            for ci, (r0, r1) in enumerate(XCH):
                for blk in range(2):
                    eng = nc.sync if (ci * 2 + blk) % 2 == 0 else nc.gpsimd
                    eng.dma_start(
                        X80[:, blk * PLANE + r0 * PITCH : blk * PLANE + r1 * PITCH],
                        xp8[0, blk, :, r0 * PITCH : r1 * PITCH],
                    )
            XR0 = xpool.tile([128, 2 * H * W], f16, tag="XR", name="XR_0")
            nc.sync.dma_start(XR0[:], xr[0, :, :])

            # h ping-pong tiles; zero the whole plane once (border rows/cols
            # are never rewritten; interior is). ScalarE Silu with scale=0
            # writes exact fp8 zeros.
            zrow = cpool.tile([128, PITCH * 2], f32, tag="zrow")
            nc.gpsimd.memset(zrow[:], 0.0)
            hts = []
            for j in range(2):
                ht = hpool.tile([128, PLANE], f8, tag=f"h{j}")
                hv = ht[:].rearrange("p (r c) -> p r c", r=HP, c=PITCH)
                for dst, n in (
                    (hv[:, 0, :], PITCH),
                    (hv[:, HP - 1, :], PITCH),
                    (hv[:, 1 : HP - 1, 0:1], HP - 2),
                    (hv[:, 1 : HP - 1, 57:58], HP - 2),
                ):
                    nc.scalar.activation(dst, zrow[:, :n], SILU, bias=0.0, scale=0.0)
                hts.append(ht)

            # PE warm-up: junk matmuls during the ~9us DMA queue-arming dead
            # time, so the HAM clock gate reaches K=8/8 (2.4GHz) before the
            # first real matmul instead of running the first ~3.4us at 1.2GHz.
            junk = cpool.tile([128, NMM], mybir.dt.float16, tag="junk")
            nc.gpsimd.memset(junk[:], 0.0)
            for _ in range(14):
                pw = pspool.tile([128, NMM], f32, tag="ps")
                nc.tensor.matmul(pw[:], junk[:, :128], junk[:], start=True, stop=True)

            for img in range(IPC):
                if img == 0:
                    X8, XR = X80, XR0
                else:
                    X8 = xpool.tile([128, 2 * PLANE], f8, tag="X8", name=f"X8_{img}")
                    XR = xpool.tile([128, 2 * H * W], f16, tag="XR", name=f"XR_{img}")
                    for blk in range(2):
                        eng = nc.sync if blk == 0 else nc.gpsimd
                        eng.dma_start(
                            X8[:, blk * PLANE : (blk + 1) * PLANE], xp8[img, blk, :, :]
                        )
                    (nc.gpsimd if img % 2 else nc.sync).dma_start(XR[:], xr[img, :, :])

                X8b = X8[:]
                XRv = XR[:].rearrange("p (b r c) -> p b r c", b=2, r=H, c=W)
                ht = hts[img % 2]
                hb = ht[:]
                hv = ht[:].rearrange("p (r c) -> p r c", r=HP, c=PITCH)

                # conv1: 256 -> 128. Weight-stationary: tap-outer over the 7
                # row-block PSUM banks; DR pairs the two cin blocks.
                ps1 = [pspool.tile([128, NMM], f32, tag="ps", name=f"ps1_{img}_{rb}") for rb in range(NRB)]
                for t in range(9):
                    ky, kx = t // 3, t % 3
                    wt = W1[:, t * 256 : (t + 1) * 256].rearrange("p (i m) -> p i m", i=2)
                    for rb in range(NRB):
                        rhs = pv(X8b, (8 * rb + ky) * PITCH + kx, [(PLANE, 2), (PITCH, 8), (1, 56)])
                        nc.tensor.matmul(
                            ps1[rb][:], wt, rhs, start=(t == 0), stop=(t == 8), perf_mode=DR
                        )
                # BN+SiLU into padded h interior (fp8 RNE write)
                for rb in range(NRB):
                    nc.scalar.activation(
                        hv[:, 1 + RB * rb : 1 + RB * rb + RB, 1:57],
                        ps1[rb][:],
                        SILU,
                        bias=AB1[:, 1:2],
                        scale=AB1[:, 0:1],
                    )

                # conv2: 128 -> 256 (two cout blocks), weight-stationary
                for cb in range(2):
                    ps2 = [pspool.tile([128, NMM], f32, tag="ps", name=f"ps2_{img}_{cb}_{rb}") for rb in range(NRB)]
                    for k, (kind, offn, pstride) in enumerate(C2PLAN):
                        base = cb * 1152 + k * 256
                        if kind == "dr":
                            wt = W2[:, base : base + 256].rearrange("p (i m) -> p i m", i=2)
                        else:
                            wt = W2[:, base : base + 128]
                        for rb in range(NRB):
                            if kind == "dr":
                                rhs = pv(hb, offn(rb), [(pstride, 2), (PITCH, 8), (1, 56)])
                                nc.tensor.matmul(
                                    ps2[rb][:], wt, rhs,
                                    start=(k == 0), stop=(k == 4), perf_mode=DR,
                                )
                            else:
                                rhs = pv(hb, offn(rb), [(PITCH, 8), (1, 56)])
                                nc.tensor.matmul(
                                    ps2[rb][:], wt, rhs, start=(k == 0), stop=(k == 4)
                                )
                    for rb in range(NRB):
                        st = opool.tile([128, NMM], f16, tag="st")
                        nc.scalar.activation(
                            st[:],
                            ps2[rb][:],
                            SILU,
                            bias=AB2[:, 2 * cb + 1 : 2 * cb + 2],
                            scale=AB2[:, 2 * cb : 2 * cb + 1],
                        )
                        ot = opool.tile([128, NMM], f16, tag="ot")
                        nc.vector.tensor_add(
                            ot[:], st[:], XRv[:, cb, RB * rb : RB * rb + RB, :]
                        )
                        # stores ride gpsimd; for the last image the other
                        # queues are idle, so spread stores to shorten the
                        # final flush the exit drain waits on
                        if img == IPC - 1:
                            seng = (nc.gpsimd, nc.sync, nc.scalar)[(rb * 2 + cb) % 3]
                        else:
                            seng = nc.gpsimd
                        seng.dma_start(y[img, cb, :, rb * NMM : (rb + 1) * NMM], ot[:])

    nc.compile()
    _CACHE["nc"] = nc
    return nc


def _quant_ternary(w):
    """Match jnp: s = max(median(|w|), Q_EPS); t = clip(round(w/s), -1, 1)."""
    w = np.asarray(w, np.float32)
    s = np.float32(np.median(np.abs(w)))
    s = np.maximum(s, np.float32(Q_EPS))
    t = np.clip(np.round(w / s), np.float32(-1.0), np.float32(1.0)).astype(np.float32)
    return s, t


def prepare_inputs(x, w1, g1, b1, m1, v1, w2, g2, b2, m2, v2):
    """Host-side prep: quantize+fold weights, pad/cast x, build per-core in_maps."""
    import ml_dtypes

    E4M3 = np.dtype(ml_dtypes.float8_e4m3fn)
    F16 = np.dtype(np.float16)

    x = np.asarray(x, np.float32)

    s1, t1 = _quant_ternary(w1)
    s2, t2 = _quant_ternary(w2)

    inv1 = np.asarray(g1, np.float32) / np.sqrt(np.asarray(v1, np.float32) + np.float32(BN_EPS))
    a1 = (s1 * inv1).astype(np.float32)
    c1 = (np.asarray(b1, np.float32) - np.asarray(m1, np.float32) * inv1).astype(np.float32)
    inv2 = np.asarray(g2, np.float32) / np.sqrt(np.asarray(v2, np.float32) + np.float32(BN_EPS))
    a2 = (s2 * inv2).astype(np.float32)
    c2 = (np.asarray(b2, np.float32) - np.asarray(m2, np.float32) * inv2).astype(np.float32)

    ab1 = np.stack([a1, c1], axis=1).astype(np.float32)  # [128, 2]
    a2b = a2.reshape(2, 128)
    c2b = c2.reshape(2, 128)
    ab2 = np.stack([a2b[0], c2b[0], a2b[1], c2b[1]], axis=1).astype(np.float32)  # [128,4]

    # conv1 DR weight tiles: [cin_p, tap, pair(blk), cout]
    # value = t1[cout, blk*128 + cin_p, ky, kx]
    w1t = (
        t1.reshape(HID, 2, 128, 3, 3)  # [m, i, p, ky, kx]
        .transpose(2, 3, 4, 1, 0)      # [p, ky, kx, i, m]
        .reshape(128, 9 * 256)
    ).astype(E4M3)

    # conv2 tiles per cout block: 3 horizontal DR pairs (ky,0)+(ky,1),
    # 1 vertical DR pair (0,2)+(1,2), 1 normal (2,2)
    t2b = t2.reshape(2, 128, 128, 3, 3)  # [cb, m, p, ky, kx]
    w2arr = np.zeros((128, 2, 1152), np.float32)  # [p, cb, cols]
    for cb in range(2):
        col = 0
        for ky in range(3):  # horizontal pairs
            for i, kx in enumerate((0, 1)):
                w2arr[:, cb, col : col + 128] = t2b[cb, :, :, ky, kx].T
                col += 128
        for i, (ky, kx) in enumerate(((0, 2), (1, 2))):  # vertical pair
            w2arr[:, cb, col : col + 128] = t2b[cb, :, :, ky, kx].T
            col += 128
        w2arr[:, cb, col : col + 128] = t2b[cb, :, :, 2, 2].T
    w2t = w2arr.reshape(128, 2 * 1152).astype(E4M3)

    # x planes: fp8 padded (pitch 64) for matmuls, fp16 unpadded for residual
    xpad = np.zeros((B, C, HP, PITCH), E4M3)
    xpad[:, :, 1 : 1 + H, 1 : 1 + W] = x.astype(E4M3)
    xp8 = xpad.reshape(NCORES, IPC, 2, 128, PLANE)
    xr = (
        x.astype(F16)
        .reshape(NCORES, IPC, 2, 128, H * W)
        .transpose(0, 1, 3, 2, 4)
        .reshape(NCORES, IPC, 128, 2 * H * W)
    )

    in_maps = []
    for c in range(NCORES):
        in_maps.append(
            {
                "xp8": np.ascontiguousarray(xp8[c]),
                "xr": np.ascontiguousarray(xr[c]),
                "w1t": w1t,
                "w2t": w2t,
                "ab1": ab1,
                "ab2": ab2,
            }
        )
    return in_maps


def assemble_output(per_core_results):
    ys = np.stack([r["y"] for r in per_core_results])  # [8, IPC, 2, 128, H*W]
    return ys.astype(np.float32).reshape(B, C, H, W)


def run_spmd(in_maps, **kwargs):
    from concourse.bass_utils import run_bass_kernel_spmd

    nc = _build_nc()
    return run_bass_kernel_spmd(nc, in_maps, core_ids=list(range(NCORES)), **kwargs)


def kernel(**inputs):
    in_maps = prepare_inputs(**inputs)
    res = run_spmd(in_maps)
    return assemble_output(res.results)
